# revision 23
# baseline (speedup 1.0000x reference)
"""AdditiveAttention (nn_AdditiveAttention_19911468385063) Trainium2 Bass kernel.

Math (per batch b):
    q = queries[b] @ Wq.T                    (nq=128, H=128)
    k = keys[b] @ Wk.T                       (nk=512, H=128)
    scores[q,k] = sum_h wv[h] * tanh(q[q,h] + k[k,h])
    attn = softmax(scores, axis=q)           # softmax over the QUERY axis
    out[b] = attn @ values[b]                (nq, dv=256)

Distribution: pure data-parallel over the batch dim B=16 -> 2 batches per
NeuronCore on 8 cores (no collectives needed). The host wrapper
pre-transposes queries/keys/weights (so the dq=256 contraction lands on
partitions with zero on-device transposes of activations) and pre-casts the
big activations to bf16.

Per-core pipeline (per batch), H=128 on partitions:
  - projections on TensorE: q_projT (H, nq) and k_projT (H, nk), fp32 PSUM
  - features: DVE tensor_scalar broadcast-adds kp + qp[:,q] (per-partition
    scalar, 4x bf16 mode) into grouped buffers; ScalarE evaluates tanh with
    large free dims (ScalarE is the bottleneck: nq*nk*H = 8.4M tanh
    elements per batch at 1 elem/lane/cycle, dtype-independent)
  - scores: "one-hot wv" stationary-operand matmuls. lhsT for query q is
    the slice B[:, 128-q:256-q] of a (128, 256) bf16 buffer whose column
    128 holds wv (zeros elsewhere), i.e. a matrix with wv in column q.
    Each matmul accumulates row q = wv . tanh_feat_q into one (nq, nk)
    fp32 PSUM tile; N=512 streaming keeps TensorE cost at ~213ns/query.
  - softmax over nq: TensorE-transpose scores into (nk, nq) tiles, exp on
    ScalarE (exp+tanh share one ACT table set), and the 1/rowsum
    normalizer is folded into the values rows (per-partition scalar mul),
    so attn itself never needs normalizing.
  - output: 4 accumulating matmuls attnT.T @ values' -> (nq, dv)

Scheduling details: ramped tanh group sizes ([1,1,2,4,8,16,24,...] on the
first batch, reversed on the last) shorten the dependency ramp and the
exposed epilogue; each batch's softmax is emitted AFTER the next batch's
score stream so its ops don't FIFO-block the pipeline on DVE/ScalarE; the
last batch uses a per-k-tile latency-optimized softmax chain with the
fused activation accumulator providing the row sums.
"""
import numpy as np
import ml_dtypes

import concourse.bass as bass
import concourse.tile as tile
from concourse import bacc, mybir
from concourse.bass_utils import run_bass_kernel_spmd
from concourse.masks import make_identity

AF = mybir.ActivationFunctionType
DT = mybir.dt

N_CORES = 8
NB = 2      # batches per core
NQ = 128
NK = 512
DQ = 256
DV = 256
H = 128
G = 24      # max queries per tanh group
NKT = NK // 128  # key tiles


def make_groups(n_act, first):
    """Group sizes summing to n_act; fine-grained at the start (first=True) or end."""
    fine = [1, 1, 2, 4, 8, 16]
    rem = n_act - sum(fine)
    body = []
    while rem > G:
        body.append(G)
        rem -= G
    if rem:
        body.append(rem)
    return fine + body if first else body[::-1] + fine[::-1]


def build(nb=NB):
    nc = bacc.Bacc(None, target_bir_lowering=False)
    qT = nc.declare_dram_parameter("qT", [nb, DQ, NQ], DT.bfloat16, isOutput=False)
    kT = nc.declare_dram_parameter("kT", [nb, DQ, NK], DT.bfloat16, isOutput=False)
    val = nc.declare_dram_parameter("val", [nb, NK, DV], DT.bfloat16, isOutput=False)
    WqT = nc.declare_dram_parameter("WqT", [DQ, H], DT.bfloat16, isOutput=False)
    WkT = nc.declare_dram_parameter("WkT", [DQ, H], DT.bfloat16, isOutput=False)
    wv = nc.declare_dram_parameter("wv", [H], DT.float32, isOutput=False)
    out = nc.declare_dram_parameter("out", [nb, NQ, DV], DT.float32, isOutput=True)

    with tile.TileContext(nc) as tc:
        with tc.tile_pool(name="const", bufs=1) as cpool, \
             tc.tile_pool(name="io", bufs=2) as io, \
             tc.tile_pool(name="work", bufs=2) as wk, \
             tc.tile_pool(name="featp", bufs=3) as featp, \
             tc.tile_pool(name="sumsp", bufs=3) as sumsp, \
             tc.tile_pool(name="pp_proj", bufs=1, space="PSUM") as pp_proj, \
             tc.tile_pool(name="pp_scores", bufs=2, space="PSUM") as pp_scores, \
             tc.tile_pool(name="pp_tr", bufs=2, space="PSUM") as pp_tr, \
             tc.tile_pool(name="pp_out", bufs=2, space="PSUM") as pp_out:

            # ---------------- prologue (once) ----------------
            warm = cpool.tile([128, 1], DT.float32)
            ident = cpool.tile([128, 128], DT.float32)
            make_identity(nc, ident[:])
            ident_b = cpool.tile([128, 128], DT.bfloat16)
            nc.vector.tensor_copy(ident_b[:], ident[:])
            WkT_s = cpool.tile([128, 2, H], DT.bfloat16)
            nc.gpsimd.dma_start(WkT_s[:], WkT[:].rearrange("(c p) h -> p c h", p=128))
            WqT_s = cpool.tile([128, 2, H], DT.bfloat16)
            nc.gpsimd.dma_start(WqT_s[:], WqT[:].rearrange("(c p) h -> p c h", p=128))
            wv_s = cpool.tile([128, 1], DT.float32)
            nc.gpsimd.dma_start(wv_s[:], wv[:].rearrange("(p one) -> p one", one=1))
            # one-hot source buffer: B[:, 128] = wv (bf16), zeros elsewhere.
            # lhsT for query q is the slice B[:, 128-q : 256-q].
            B = cpool.tile([128, 256], DT.bfloat16)
            nc.vector.memset(B[:], 0.0)
            nc.vector.tensor_copy(B[:, 128:129], wv_s[:])

            def softmax_out(b, ps_sc, val_s, latency):
                ps_o = pp_out.tile([128, DV], DT.float32, tag="ps_o")
                if not latency:
                    # throughput version: batched ops
                    sc = wk.tile([128, NK], DT.float32, tag="sc")
                    nc.vector.tensor_copy(sc[:], ps_sc[:])
                    scT_ps = pp_tr.tile([128, NKT, 128], DT.float32, tag="scT_ps")
                    for t in range(NKT):
                        nc.tensor.transpose(scT_ps[:, t, :], sc[:, t * 128:(t + 1) * 128],
                                            ident[:])
                    expT = wk.tile([128, NKT, NQ], DT.bfloat16, tag="expT")
                    nc.scalar.activation(expT[:].rearrange("p a b -> p (a b)"),
                                         scT_ps[:].rearrange("p a b -> p (a b)"), AF.Exp)
                    ssum = wk.tile([128, NKT], DT.float32, tag="ssum")
                    nc.vector.reduce_sum(ssum[:], expT[:], axis=mybir.AxisListType.X)
                    rec = wk.tile([128, NKT], DT.float32, tag="rec")
                    nc.vector.reciprocal(rec[:], ssum[:])
                    valn = wk.tile([128, NKT, DV], DT.bfloat16, tag="valn")
                    for t in range(NKT):
                        nc.vector.tensor_scalar_mul(valn[:, t, :], val_s[:, t, :],
                                                    rec[:, t:t + 1])
                    for t in range(NKT):
                        nc.tensor.matmul(ps_o[:], expT[:, t, :], valn[:, t, :],
                                         start=(t == 0), stop=(t == NKT - 1))
                else:
                    # latency version for the exposed last batch: skewed per-tile
                    # chain, copies running one tile ahead of the rest
                    sc = wk.tile([128, NK], DT.float32, tag="sc")
                    expT = wk.tile([128, NKT, NQ], DT.bfloat16, tag="expT")
                    rec = wk.tile([128, NKT], DT.float32, tag="rec")
                    valn = wk.tile([128, NKT, DV], DT.bfloat16, tag="valn")
                    scT_ps = pp_tr.tile([128, NKT, 128], DT.float32, tag="scT_ps")
                    ssum = wk.tile([128, NKT], DT.float32, tag="ssum")
                    for t in range(NKT + 2):
                        if t < NKT:
                            nc.vector.tensor_copy(sc[:, t * 128:(t + 1) * 128],
                                                  ps_sc[:, t * 128:(t + 1) * 128])
                        if t >= 2:
                            u = t - 2
                            nc.tensor.transpose(scT_ps[:, u, :],
                                                sc[:, u * 128:(u + 1) * 128], ident[:])
                            nc.scalar.activation(expT[:, u, :], scT_ps[:, u, :], AF.Exp,
                                                 accum_out=ssum[:, u:u + 1])
                            nc.vector.reciprocal(rec[:, u:u + 1], ssum[:, u:u + 1])
                            nc.vector.tensor_scalar_mul(valn[:, u, :], val_s[:, u, :],
                                                        rec[:, u:u + 1])
                            nc.tensor.matmul(ps_o[:], expT[:, u, :], valn[:, u, :],
                                             start=(u == 0), stop=(u == NKT - 1))
                out_s = wk.tile([128, DV], DT.float32, tag="out_s")
                nc.vector.tensor_copy(out_s[:], ps_o[:])
                nc.sync.dma_start(out[b], out_s[:])

            prev = None
            for b in range(nb):
                # ---------------- loads ----------------
                kT_s = io.tile([128, 2, NK], DT.bfloat16, tag="kT_s")
                nc.sync.dma_start(kT_s[:, 0, :], kT[b, 0:128, :])
                # batch 0: second half on the (idle) ACT queue -> parallel DMA rings
                (nc.scalar if b == 0 else nc.sync).dma_start(
                    kT_s[:, 1, :], kT[b, 128:256, :])
                qT_s = io.tile([128, 2, NQ], DT.bfloat16, tag="qT_s")
                nc.gpsimd.dma_start(qT_s[:], qT[b].rearrange("(c p) n -> p c n", p=128))
                if b == 0:
                    # warm the ACT table (exp_and_others holds tanh+exp) now --
                    # after the DMA issues so they hit the wire first
                    nc.vector.memset(warm[:], 0.0)
                    nc.scalar.activation(warm[:], warm[:], AF.Tanh)

                # ---------------- projections ----------------
                ps_q = pp_proj.tile([128, NQ], DT.float32, tag="ps_q")
                for c in range(2):
                    nc.tensor.matmul(ps_q[:], WqT_s[:, c, :], qT_s[:, c, :],
                                     start=(c == 0), stop=(c == 1))
                qp = wk.tile([128, NQ], DT.float32, tag="qp")
                nc.vector.tensor_copy(qp[:], ps_q[:])

                ps_k = pp_proj.tile([128, NK], DT.float32, tag="ps_k")
                for c in range(2):
                    nc.tensor.matmul(ps_k[:], WkT_s[:, c, :], kT_s[:, c, :],
                                     start=(c == 0), stop=(c == 1))
                kp = wk.tile([128, NK], DT.bfloat16, tag="kp")
                if b == 0:
                    nc.scalar.copy(kp[:], ps_k[:])  # ACT idle; DVE copies qp meanwhile
                else:
                    nc.vector.tensor_copy(kp[:], ps_k[:])

                # ---------------- features + scores ----------------
                GROUPS = make_groups(NQ, first=(b == 0))
                ps_sc = pp_scores.tile([128, NK], DT.float32, tag="ps_sc")
                q0 = 0
                for gsz in GROUPS:
                    sums = sumsp.tile([128, G, NK], DT.bfloat16, tag="sums")
                    for j in range(gsz):
                        nc.vector.tensor_scalar_add(sums[:, j, :], kp[:],
                                                    qp[:, q0 + j:q0 + j + 1])
                    feat = featp.tile([128, G, NK], DT.bfloat16, tag="feat")
                    nc.scalar.activation(
                        feat[:, :gsz, :].rearrange("p a b -> p (a b)"),
                        sums[:, :gsz, :].rearrange("p a b -> p (a b)"), AF.Tanh)
                    for j in range(gsz):
                        q = q0 + j
                        nc.tensor.matmul(ps_sc[:], B[:, 128 - q:256 - q], feat[:, j, :],
                                         start=(q == 0), stop=(q == NQ - 1))
                    q0 += gsz

                # values load (needed only for the epilogue)
                val_s = io.tile([128, NKT, DV], DT.bfloat16, tag="val_s")
                nc.gpsimd.dma_start(val_s[:], val[b].rearrange("(t p) v -> p t v", p=128))

                # previous batch's softmax+output AFTER this batch's score stream,
                # so its DVE/PE/ACT ops don't FIFO-block this batch's pipeline.
                if prev is not None:
                    softmax_out(*prev, latency=False)
                prev = (b, ps_sc, val_s)

            softmax_out(*prev, latency=True)

    nc.finalize()
    return nc


def make_in_maps(queries, keys, values, Wq, Wk, wv, n_cores=N_CORES):
    bf16 = ml_dtypes.bfloat16
    B = queries.shape[0]
    nb = B // n_cores
    qT = np.ascontiguousarray(np.transpose(queries, (0, 2, 1))).astype(bf16)
    kT = np.ascontiguousarray(np.transpose(keys, (0, 2, 1))).astype(bf16)
    val = np.ascontiguousarray(values).astype(bf16)
    WqT_a = np.ascontiguousarray(np.asarray(Wq).T).astype(bf16)
    WkT_a = np.ascontiguousarray(np.asarray(Wk).T).astype(bf16)
    wv_a = np.ascontiguousarray(wv).astype(np.float32)
    maps = []
    for c in range(n_cores):
        s = slice(c * nb, (c + 1) * nb)
        maps.append({
            "qT": qT[s], "kT": kT[s], "val": val[s],
            "WqT": WqT_a, "WkT": WkT_a, "wv": wv_a,
        })
    return maps


_nc_cache = {}


def _get_nc(nb=NB):
    if nb not in _nc_cache:
        _nc_cache[nb] = build(nb)
    return _nc_cache[nb]


def kernel(queries, keys, values, Wq, Wk, wv):
    queries = np.asarray(queries, dtype=np.float32)
    keys = np.asarray(keys, dtype=np.float32)
    values = np.asarray(values, dtype=np.float32)
    Wq = np.asarray(Wq, dtype=np.float32)
    Wk = np.asarray(Wk, dtype=np.float32)
    wv = np.asarray(wv, dtype=np.float32)

    B = queries.shape[0]
    nb = B // N_CORES
    nc = _get_nc(nb)
    maps = make_in_maps(queries, keys, values, Wq, Wk, wv, N_CORES)
    res = run_bass_kernel_spmd(nc, maps, core_ids=list(range(N_CORES)))
    out = np.concatenate(
        [np.asarray(res.results[i]["out"]).reshape(nb, NQ, DV) for i in range(N_CORES)],
        axis=0)
    return out.astype(np.float32)
